# revision 6
# baseline (speedup 1.0000x reference)
"""TRN2 Bass kernel for nn_Attention: fused 3-linear projections + softmax attention.

Strategy:
  - Data-parallel over batch B=8 across 8 NeuronCores (no collectives).
  - Host: collapse each projection chain w1@w2@w3 -> [1024, 64] (associativity),
    pre-round all device inputs to tf32 so float32r matmuls are bit-exact on
    the rounded values.
  - Device (per core, one batch element):
      Phase A: PE-transpose x tiles -> xT, project to qT/kT [64,2048] and
               v (natural, augmented with a ones column for fused row-sums).
      Phase B: dotsT = k @ qT (transposed layout), exp on ACT,
               attn@v + softmax-sums in one matmul via [v|ones] lhsT,
               PE-transpose exp(dots)T back to natural + scale by 1/sum -> attn,
               out = (attn@v scaled) @ wo + bo via [.;ones] lhsT / [wo;bo] rhs.
"""
import numpy as np

import concourse.tile as tile
from concourse import bacc, mybir
from concourse.bass_utils import run_bass_kernel_spmd

F32 = mybir.dt.float32
F32R = mybir.dt.float32r

B, NQ, NK, DIM, DH = 8, 2048, 2048, 1024, 64
NCHUNK = DIM // 128          # 8 contraction chunks
NBLK = NQ // 512             # 4 row blocks (phase A) / q blocks (phase B)
NKC = NK // 128              # 16 key chunks
ExpF = mybir.ActivationFunctionType.Exp
CopyF = mybir.ActivationFunctionType.Copy


def tf32_round(x: np.ndarray) -> np.ndarray:
    """Round f32 mantissa to 10 bits (tf32), round-half-up. Makes float32r
    matmuls bit-exact on the stored values."""
    u = x.astype(np.float32).view(np.uint32).astype(np.uint64)
    r = (u + 0x1000) & np.uint64(0xFFFFE000)
    return r.astype(np.uint32).view(np.float32)


def build_nc():
    nc = bacc.Bacc("TRN2", target_bir_lowering=False, debug=False)

    xq_d = nc.declare_dram_parameter("xq", [NQ, DIM], F32R, isOutput=False)
    xk_d = nc.declare_dram_parameter("xk", [NK, DIM], F32R, isOutput=False)
    xv_d = nc.declare_dram_parameter("xv", [NK, DIM], F32R, isOutput=False)
    w_d = nc.declare_dram_parameter("w_all", [128, NCHUNK, 3 * DH], F32R, isOutput=False)
    wo_d = nc.declare_dram_parameter("wo_b", [DH + 1, DIM], F32R, isOutput=False)
    id_d = nc.declare_dram_parameter("ident", [128, 128], F32R, isOutput=False)
    ones_d = nc.declare_dram_parameter("ones", [128, 512], F32R, isOutput=False)
    out_d = nc.declare_dram_parameter("out", [NQ, DIM], F32, isOutput=True)
    attn_d = nc.declare_dram_parameter("attn", [NQ, NK], F32, isOutput=True)

    with tile.TileContext(nc) as tc:
        with (
            tc.tile_pool(name="const", bufs=1) as constp,
            tc.tile_pool(name="persist", bufs=1) as persist,
        ):
            w_s = constp.tile([128, NCHUNK, 3 * DH], F32R)
            nc.sync.dma_start(out=w_s, in_=w_d[:, :, :])
            wo_s = constp.tile([DH + 1, DIM], F32R)
            nc.sync.dma_start(out=wo_s, in_=wo_d[:, :])
            id_s = constp.tile([128, 128], F32R)
            nc.sync.dma_start(out=id_s, in_=id_d[:, :])
            ones_s = constp.tile([128, 512], F32R)
            nc.sync.dma_start(out=ones_s, in_=ones_d[:, :])

            qT_s = persist.tile([DH, NQ], F32R)    # q^T, partitions 0..63
            kT_s = persist.tile([DH, NK], F32R)    # k^T
            v_aug = persist.tile([128, NKC, DH + 1], F32R)  # [v | ones] per k-chunk
            nc.sync.dma_start(out=v_aug[:, :, DH:DH + 1], in_=ones_d[:, 0:NKC])

            # ---------------- Phase A: transpose x, project ----------------
            with (
                tc.tile_pool(name="xin", bufs=2) as xin,
                tc.tile_pool(name="xT", bufs=1) as xTp,
                tc.tile_pool(name="vst", bufs=2) as vstp,
                tc.tile_pool(name="ps_xt", bufs=3, space="PSUM") as ps_xt,
                tc.tile_pool(name="ps_q", bufs=2, space="PSUM") as ps_q,
                tc.tile_pool(name="ps_k", bufs=1, space="PSUM") as ps_k,
                tc.tile_pool(name="ps_v", bufs=1, space="PSUM") as ps_v,
                tc.tile_pool(name="ps_vn", bufs=1, space="PSUM") as ps_vn,
            ):
                ncopy = 0
                for blk in range(NBLK):
                    xTs = {}
                    for name, x_d in (("q", xq_d), ("k", xk_d), ("v", xv_d)):
                        xT_t = xTp.tile([128, NCHUNK, 4, 128], F32R, tag=f"xT{name}")
                        xTs[name] = xT_t
                        for rt in range(4):
                            r0 = (blk * 4 + rt) * 128
                            x_t = xin.tile([128, DIM], F32R, tag=f"x{name}")
                            nc.sync.dma_start(out=x_t, in_=x_d[r0:r0 + 128, :])
                            for h in range(2):
                                pst = ps_xt.tile([128, 4, 128], F32R)
                                for j in range(4):
                                    d = 4 * h + j
                                    nc.tensor.transpose(
                                        pst[:, j, :], x_t[:, d * 128:(d + 1) * 128], id_s)
                                dst = xT_t[:, 4 * h:4 * h + 4, rt, :]
                                if ncopy % 2 == 0:
                                    nc.vector.tensor_copy(dst, pst)
                                else:
                                    nc.scalar.copy(dst, pst)
                                ncopy += 1
                    # projections for this 512-row block
                    c0 = blk * 512
                    q_ps = ps_q.tile([DH, 512], F32)
                    k_ps = ps_k.tile([DH, 512], F32)
                    v_ps = ps_v.tile([DH, 512], F32)
                    for d in range(NCHUNK):
                        st, sp = d == 0, d == NCHUNK - 1
                        nc.tensor.matmul(q_ps, w_s[:, d, 0:DH],
                                         xTs["q"][:, d, :, :], start=st, stop=sp)
                        nc.tensor.matmul(k_ps, w_s[:, d, DH:2 * DH],
                                         xTs["k"][:, d, :, :], start=st, stop=sp)
                        nc.tensor.matmul(v_ps, w_s[:, d, 2 * DH:3 * DH],
                                         xTs["v"][:, d, :, :], start=st, stop=sp)
                    nc.vector.tensor_copy(qT_s[:, c0:c0 + 512], q_ps)
                    nc.scalar.copy(kT_s[:, c0:c0 + 512], k_ps)
                    vst = vstp.tile([DH, 512], F32R)
                    nc.vector.tensor_copy(vst, v_ps)
                    vn_ps = ps_vn.tile([128, 4, DH], F32R)
                    for rt in range(4):
                        nc.tensor.transpose(
                            vn_ps[:, rt, :], vst[:, rt * 128:(rt + 1) * 128],
                            id_s[0:DH, 0:DH])
                    nc.vector.tensor_copy(
                        v_aug[:, blk * 4:blk * 4 + 4, 0:DH], vn_ps)

            # ---------------- Phase B: attention ----------------
            with (
                tc.tile_pool(name="ed", bufs=1) as edp,
                tc.tile_pool(name="att", bufs=2) as attp,
                tc.tile_pool(name="outs", bufs=2) as outsp,
                tc.tile_pool(name="smal", bufs=2) as smal,
                tc.tile_pool(name="ps_d", bufs=2, space="PSUM") as ps_d,
                tc.tile_pool(name="ps_av", bufs=1, space="PSUM") as ps_av,
                tc.tile_pool(name="ps_bc", bufs=1, space="PSUM") as ps_bc,
                tc.tile_pool(name="ps_tr", bufs=2, space="PSUM") as ps_tr,
                tc.tile_pool(name="ps_o", bufs=1, space="PSUM") as ps_o,
            ):
                nsc = 0
                for qb in range(NBLK):
                    q0 = qb * 512
                    eks = []
                    for kc in range(NKC):
                        d_ps = ps_d.tile([128, 512], F32)
                        nc.tensor.matmul(
                            d_ps, kT_s[:, kc * 128:(kc + 1) * 128],
                            qT_s[:, q0:q0 + 512], start=True, stop=True)
                        e_k = edp.tile([128, 512], F32R, tag=f"e{kc}")
                        nc.scalar.activation(out=e_k, in_=d_ps, func=ExpF)
                        eks.append(e_k)
                    # attn @ v with fused row-sums (ones column)
                    av_ps = ps_av.tile([DH + 1, 512], F32)
                    for kc in range(NKC):
                        nc.tensor.matmul(av_ps, v_aug[:, kc, :], eks[kc],
                                         start=(kc == 0), stop=(kc == NKC - 1))
                    # 1/sum at partition DH
                    recip = smal.tile([DH + 1, 512], F32R, tag="recip")
                    with nc.allow_low_precision(reason="tf32 recip is within error budget"):
                        nc.vector.reciprocal(recip[DH:DH + 1, :], av_ps[DH:DH + 1, :])
                    # broadcast recip across partitions 0..63 via PE outer product
                    bc_ps = ps_bc.tile([128, 512], F32, tag="bcrt")
                    nc.tensor.matmul(bc_ps, ones_s[DH:DH + 1, 0:128],
                                     recip[DH:DH + 1, :], start=True, stop=True)
                    bc = smal.tile([128, 512], F32R, tag="bc")
                    nc.scalar.copy(bc, bc_ps)
                    av_s = smal.tile([DH, 512], F32R, tag="avs")
                    nc.vector.tensor_copy(av_s, av_ps[0:DH, :])
                    lhsT_av = smal.tile([DH + 1, 512], F32R, tag="lav")
                    nc.vector.tensor_mul(lhsT_av[0:DH, :], av_s, bc[0:DH, :])
                    nc.vector.tensor_copy(lhsT_av[DH:DH + 1, :], ones_s[DH:DH + 1, :])

                    for qs in range(4):
                        qg = q0 + qs * 128
                        # 1/sum as a per-partition column: 32x32 DVE block
                        # transposes of bc (whose columns are constant down
                        # partitions) yield recip[qg+p] at partition p.
                        scratch = smal.tile([128, 32], F32, tag="rT_s")
                        for m in range(4):
                            c0 = qs * 128 + 32 * m
                            nc.vector.transpose(
                                scratch[32 * m:32 * m + 32, :],
                                bc[32 * m:32 * m + 32, c0:c0 + 32])
                        recipT = scratch[:, 0:1]
                        att_n = attp.tile([128, NK], F32)
                        for bank in range(4):
                            t_ps = ps_tr.tile([128, 4, 128], F32R)
                            for j in range(4):
                                kc = bank * 4 + j
                                nc.tensor.transpose(
                                    t_ps[:, j, :],
                                    eks[kc][:, qs * 128:(qs + 1) * 128], id_s)
                            dst = att_n[:, bank * 512:(bank + 1) * 512]
                            if nsc % 2 == 0:
                                nc.vector.tensor_scalar_mul(dst, t_ps, recipT)
                            else:
                                nc.scalar.activation(out=dst, in_=t_ps, func=CopyF,
                                                     scale=recipT)
                            nsc += 1
                        nc.sync.dma_start(out=attn_d[qg:qg + 128, :], in_=att_n)
                        # output projection: [scaled av; ones]^T @ [wo; bo]
                        o_ps = ps_o.tile([128, 2, 512], F32)
                        for h in range(2):
                            nc.tensor.matmul(
                                o_ps[:, h, :],
                                lhsT_av[:, qs * 128:(qs + 1) * 128],
                                wo_s[:, h * 512:(h + 1) * 512],
                                start=True, stop=True)
                        out_s = outsp.tile([128, DIM], F32)
                        nc.vector.tensor_copy(out_s[:, 0:512], o_ps[:, 0, :])
                        nc.scalar.copy(out_s[:, 512:1024], o_ps[:, 1, :])
                        nc.sync.dma_start(out=out_d[qg:qg + 128, :], in_=out_s)

    nc.compile()
    return nc


_CACHE = {}


def _get_nc():
    if "nc" not in _CACHE:
        _CACHE["nc"] = build_nc()
    return _CACHE["nc"]


def _host_inputs(query, key, value, wq1, wq2, wq3, wk1, wk2, wk3,
                 wv1, wv2, wv3, wo, bo):
    f64 = np.float64
    Wq = (wq1.astype(f64) @ wq2.astype(f64) @ wq3.astype(f64)).astype(np.float32)
    Wk = (wk1.astype(f64) @ wk2.astype(f64) @ wk3.astype(f64)).astype(np.float32)
    Wv = (wv1.astype(f64) @ wv2.astype(f64) @ wv3.astype(f64)).astype(np.float32)
    w_all = np.empty((128, NCHUNK, 3 * DH), np.float32)
    for c in range(NCHUNK):
        w_all[:, c, 0:DH] = Wq[c * 128:(c + 1) * 128, :]
        w_all[:, c, DH:2 * DH] = Wk[c * 128:(c + 1) * 128, :]
        w_all[:, c, 2 * DH:3 * DH] = Wv[c * 128:(c + 1) * 128, :]
    wo_b = np.concatenate([np.asarray(wo, np.float32),
                           np.asarray(bo, np.float32)[None, :]], axis=0)
    w_all = tf32_round(w_all)
    wo_b = tf32_round(wo_b)
    ident = np.eye(128, dtype=np.float32)
    ones_np = np.ones((128, 512), dtype=np.float32)
    query = tf32_round(np.ascontiguousarray(query))
    key = tf32_round(np.ascontiguousarray(key))
    value = tf32_round(np.ascontiguousarray(value))
    in_maps = []
    for b in range(B):
        in_maps.append({
            "xq": query[b], "xk": key[b], "xv": value[b],
            "w_all": w_all, "wo_b": wo_b, "ident": ident, "ones": ones_np,
        })
    return in_maps


def kernel(**inputs):
    nc = _get_nc()
    in_maps = _host_inputs(**inputs)
    res = run_bass_kernel_spmd(nc, in_maps, list(range(B)), trace=False)
    out = np.stack([res.results[b]["out"] for b in range(B)], axis=0)
    attn = np.stack([res.results[b]["attn"] for b in range(B)], axis=0)
    return out, attn
